# revision 16
# baseline (speedup 1.0000x reference)
"""Trainium2 Bass kernel for DistanceGumbelSoftmaxVQ.

Reference reduces to (forward numerics):
  d2(n,k)  = ||x_n||^2 + ||c_k||^2 - 2 x_n.c_k
  z(n,k)   = gumbel(n,k) - sqrt(d2)          (gumbel: fixed key(42) tensor)
  idx_n    = argmax_k z
  codes    = one_hot(idx)                     (y_soft terms cancel exactly)
  loss     = 1.25 * mean_n d2(n, idx_n) / 1  ... = 1.25*mean((cb[idx]-x)^2)
  new_cb   = 0.99*cb + 0.01 * codes.T @ x

Sharding: data-parallel over N rows across 8 cores; codebook replicated.
Device per core: dist matmul (PE) -> sqrt w/ fused affine (ACT) ->
z = g - dist (DVE/GPSIMD split) -> row top-8 (DVE Max8) ->
one-hot via is_equal (GPSIMD) -> codes out; codes^T@x and counts
accumulated on PE into PSUM across all row tiles.

Host: gathers shards, sums partial cbsum/counts, assembles the loss from
counts/cbsum/c2/x2 (all idx-dependent terms), and exactly recomputes the
few rows whose top-2 gap is below a threshold (device sqrt is a ~1e-4-abs
spline approx; ambiguous rows are patched from f32-exact host values so
the argmax matches the reference bit-for-bit).
"""

import numpy as np

N, D, K = 65536, 128, 1024
NCORES = 8
R = N // NCORES            # rows per core
P = 128                    # partitions / rows per tile
SUB = 4                    # row-tiles per DMA chunk
DECAY = 0.99
COMMIT = 0.25
GAP_THRESHOLD = 5e-3       # host-correct rows with top2 gap below this

# fraction of the z = g - dist subtraction done on DVE (rest on GPSIMD)
ZSPLIT = 640

_CACHE = {}
TRACE = False          # set by test harness to collect a HW profile
LAST_RESULTS = None    # BassKernelResults of the last run (for profiling)


def _build_nc(rows):
    import concourse.bass as bass
    import concourse.mybir as mybir
    from concourse.tile import TileContext

    # --- walrus workaround: split tail-drain waits into single-wait NOPs ---
    from concourse.tile import TileContext as _TC
    from concourse.vector_clock import ScopedClock as _SC

    def _patched_drain(self, tick_clock, wait_clock):
        nc = self.nc
        probe = nc.sync.nop()
        wait_clock.add_sem_waits(probe.ins, _SC({None: tick_clock.global_clock}))
        si = probe.ins.sync_info
        waits = list(si.on_wait or []) if si is not None else []
        if si is not None:
            probe.ins.sync_info = mybir.SyncInfo(
                on_wait=waits[:1], on_update=list(si.on_update or [])
            )
        for w in waits[1:]:
            n2 = nc.sync.nop()
            n2.ins.sync_info = mybir.SyncInfo(on_wait=[w], on_update=[])
        nc.sync.drain()
        nc.all_engine_barrier()
        assert self.sems is not None
        popped = nc._tile_sem_poison_stack.pop()
        assert popped is self._sem_poison
        nc.clear_and_free_semaphores(list(self.sems.allocated().values()))
        nc.all_engine_barrier()

    _TC._drain_and_barrier = _patched_drain

    # --- walrus workaround #2: this compiler rejects instructions carrying
    # more than MAX_WAITS sync waits; hoist the excess onto same-engine NOPs
    # placed immediately before the instruction. ---
    MAX_WAITS = 1

    def _split_excess_waits(nc):
        n_split = 0
        for bbb in nc.bb_map.values():
            insts = bbb.bb.instructions
            out = []
            changed = False
            for inst in insts:
                si = getattr(inst, "sync_info", None)
                waits = list(si.on_wait or []) if si is not None else []
                if len(waits) > MAX_WAITS:
                    extra, keep = waits[:-MAX_WAITS], waits[-MAX_WAITS:]
                    for j in range(0, len(extra), MAX_WAITS):
                        nop = mybir.InstNoOp(
                            name=f"{inst.name}-wsplit-{j}",
                            sync_info=mybir.SyncInfo(
                                on_wait=extra[j:j + MAX_WAITS], on_update=[]
                            ),
                            engine=inst.engine,
                            bass_nofuse=True,
                        )
                        nc.register_instruction(nop, overwrite=True)
                        out.append(nop)
                        n_split += 1
                    inst.sync_info = mybir.SyncInfo(
                        on_wait=keep, on_update=list(si.on_update or [])
                    )
                    changed = True
                out.append(inst)
            if changed:
                insts.clear()
                insts.extend(out)
        return n_split

    f32 = mybir.dt.float32
    T = rows // P              # row tiles
    NCH = T // SUB             # DMA chunks

    nc = bass.Bass()
    x_d = nc.declare_dram_parameter("x", [rows, D], f32, isOutput=False)
    xt_d = nc.declare_dram_parameter("xt", [D, rows], f32, isOutput=False)
    x2_d = nc.declare_dram_parameter("x2", [P, T], f32, isOutput=False)
    g_d = nc.declare_dram_parameter("g", [rows, K], f32, isOutput=False)
    cbt_d = nc.declare_dram_parameter("cbt", [D, K], f32, isOutput=False)
    c2h_d = nc.declare_dram_parameter("c2h", [1, K], f32, isOutput=False)
    codes_d = nc.declare_dram_parameter("codes", [rows, K], f32, isOutput=True)
    cbsum_d = nc.declare_dram_parameter("cbsumT", [D, K], f32, isOutput=True)
    counts_d = nc.declare_dram_parameter("counts", [1, K], f32, isOutput=True)
    maxs_d = nc.declare_dram_parameter("maxs", [P, 2 * T], f32, isOutput=True)

    g_ap = g_d.rearrange("(c s p) k -> c p s k", p=P, s=SUB)
    codes_ap = codes_d.rearrange("(c s p) k -> c p s k", p=P, s=SUB)
    x_ap = x_d.rearrange("(t p) d -> p t d", p=P)

    with TileContext(nc) as tc:
        with (
            tc.tile_pool(name="persist", bufs=1) as pp,
            tc.tile_pool(name="gpool", bufs=2) as gp,
            tc.tile_pool(name="cpool", bufs=2) as cp,
            tc.tile_pool(name="work", bufs=3) as wp,
            tc.tile_pool(name="pd", bufs=2, space="PSUM") as pdp,
            tc.tile_pool(name="pacc", bufs=1, space="PSUM") as pap,
        ):
            # persistent loads
            x_sb = pp.tile([P, T, D], f32)
            nc.sync.dma_start(out=x_sb[:], in_=x_ap[:])
            xt_sb = pp.tile([D, rows], f32)
            nc.sync.dma_start(out=xt_sb[:], in_=xt_d[:])
            x2_sb = pp.tile([P, T], f32)
            nc.sync.dma_start(out=x2_sb[:], in_=x2_d[:])
            cbt_sb = pp.tile([D, K], f32)
            nc.sync.dma_start(out=cbt_sb[:], in_=cbt_d[:])
            c2h_sb = pp.tile([1, K], f32)
            nc.sync.dma_start(out=c2h_sb[:], in_=c2h_d[:])
            onesrow = pp.tile([1, P], f32)
            nc.vector.memset(onesrow[:], 1.0)
            onescol = pp.tile([P, 1], f32)
            nc.vector.memset(onescol[:], 1.0)
            maxs_sb = pp.tile([P, 2 * T], f32)

            pcbT = pap.tile([D, K], f32)        # x^T @ codes accumulator (cbsum^T)
            pcnt0 = pap.tile([1, 512], f32)
            pcnt1 = pap.tile([1, 512], f32)
            pcnt = [pcnt0, pcnt1]

            for c in range(NCH):
                g_sb = gp.tile([P, SUB, K], f32, tag="g")
                nc.sync.dma_start(out=g_sb[:], in_=g_ap[c])
                codes_sb = cp.tile([P, SUB, K], f32, tag="codes")
                for s in range(SUB):
                    t = c * SUB + s
                    pd = pdp.tile([P, K], f32, tag="pd")
                    for h in range(2):
                        ksl = slice(h * 512, (h + 1) * 512)
                        nc.tensor.matmul(
                            pd[:, ksl],
                            lhsT=xt_sb[:, t * P:(t + 1) * P],
                            rhs=cbt_sb[:, ksl],
                            start=True, stop=False,
                        )
                        nc.tensor.matmul(
                            pd[:, ksl],
                            lhsT=onesrow[:1, :],
                            rhs=c2h_sb[:1, ksl],
                            start=False, stop=True,
                        )
                    # dist = sqrt(x2 - 2*pd)   (pd = x.c - 0.5*||c||^2)
                    dist = wp.tile([P, K], f32, tag="dist")
                    nc.scalar.activation(
                        dist[:], pd[:],
                        mybir.ActivationFunctionType.Sqrt,
                        bias=x2_sb[:, t:t + 1], scale=-2.0,
                    )
                    # z = g - dist  (split DVE / GPSIMD)
                    z = wp.tile([P, K], f32, tag="z")
                    gs = g_sb[:, s, :]
                    if ZSPLIT > 0:
                        nc.vector.tensor_sub(z[:, :ZSPLIT], gs[:, :ZSPLIT], dist[:, :ZSPLIT])
                    if ZSPLIT < K:
                        nc.gpsimd.tensor_sub(z[:, ZSPLIT:], gs[:, ZSPLIT:], dist[:, ZSPLIT:])
                    # top-8 per row
                    m8 = wp.tile([P, 8], f32, tag="m8")
                    nc.vector.max(out=m8[:], in_=z[:])
                    nc.scalar.copy(out=maxs_sb[:, 2 * t:2 * t + 2], in_=m8[:, 0:2])
                    # one-hot: codes = (z == rowmax)
                    nc.gpsimd.tensor_scalar(
                        codes_sb[:, s, :], z[:], m8[:, 0:1], None,
                        op0=mybir.AluOpType.is_equal,
                    )
                    # accumulate x^T @ codes and ones^T @ codes (counts)
                    for h in range(2):
                        ksl = slice(h * 512, (h + 1) * 512)
                        nc.tensor.matmul(
                            pcbT[:, ksl], lhsT=x_sb[:, t, :],
                            rhs=codes_sb[:, s, ksl],
                            start=(t == 0), stop=(t == T - 1),
                        )
                        nc.tensor.matmul(
                            pcnt[h][:, :], lhsT=onescol[:],
                            rhs=codes_sb[:, s, ksl],
                            start=(t == 0), stop=(t == T - 1),
                        )
                nc.sync.dma_start(out=codes_ap[c], in_=codes_sb[:])

            cbs_sb = pp.tile([D, K], f32)
            nc.scalar.copy(out=cbs_sb[:], in_=pcbT[:])
            nc.sync.dma_start(out=cbsum_d[:], in_=cbs_sb[:])
            cnt_sb = pp.tile([1, K], f32)
            for h in range(2):
                nc.scalar.copy(
                    out=cnt_sb[:, h * 512:(h + 1) * 512], in_=pcnt[h][:, :]
                )
            nc.sync.dma_start(out=counts_d[:], in_=cnt_sb[:])
            nc.sync.dma_start(out=maxs_d[:], in_=maxs_sb[:])

    _split_excess_waits(nc)
    return nc


def _host_prep(inputs_np, codebook_np):
    """Host-side constant/derived tensors. Gumbel must match
    jax.random.gumbel(key(42), (N,K), float32) bit-for-bit."""
    if "g" not in _CACHE:
        import jax
        import jax.numpy as jnp
        g = np.asarray(jax.random.gumbel(jax.random.key(42), (N, K), jnp.float32))
        _CACHE["g"] = g
    g = _CACHE["g"]
    x = np.ascontiguousarray(inputs_np, dtype=np.float32)
    cb = np.ascontiguousarray(codebook_np, dtype=np.float32)
    cbt = np.ascontiguousarray(cb.T)
    c2 = (cb.astype(np.float64) ** 2).sum(1)
    c2h = np.ascontiguousarray((-0.5 * c2).astype(np.float32)[None, :])
    x2 = (x.astype(np.float64) ** 2).sum(1).astype(np.float32)
    return g, x, cb, cbt, c2, c2h, x2


def build_inmaps(x, g, cbt, c2h, x2):
    T = R // P
    in_maps = []
    for i in range(NCORES):
        rs = slice(i * R, (i + 1) * R)
        xs = x[rs]
        in_maps.append({
            "x": xs,
            "xt": np.ascontiguousarray(xs.T),
            "x2": np.ascontiguousarray(x2[rs].reshape(T, P).T),
            "g": g[rs],
            "cbt": cbt,
            "c2h": c2h,
        })
    return in_maps


def get_nc():
    if "nc" not in _CACHE:
        _CACHE["nc"] = _build_nc(R)
    return _CACHE["nc"]


def kernel(inputs, codebook):
    from concourse.bass_utils import run_bass_kernel_spmd

    g, x, cb, cbt, c2, c2h, x2 = _host_prep(np.asarray(inputs), np.asarray(codebook))
    T = R // P
    nc = get_nc()
    in_maps = build_inmaps(x, g, cbt, c2h, x2)

    res = run_bass_kernel_spmd(nc, in_maps, list(range(NCORES)), trace=TRACE)
    global LAST_RESULTS
    LAST_RESULTS = res

    codes = np.empty((N, K), dtype=np.float32)
    cbsum = np.zeros((K, D), dtype=np.float64)
    counts = np.zeros(K, dtype=np.float64)
    gaps = np.empty(N, dtype=np.float32)
    for i in range(NCORES):
        r = res.results[i]
        rs = slice(i * R, (i + 1) * R)
        codes[rs] = r["codes"]
        cbsum += r["cbsumT"].T.astype(np.float64)
        counts += r["counts"].reshape(K).astype(np.float64)
        m = r["maxs"].reshape(P, T, 2)
        gaps[rs] = (m[:, :, 0] - m[:, :, 1]).T.reshape(R)

    # ---- exact host correction of near-tie rows ----
    x64 = x.astype(np.float64)
    cb64 = cb.astype(np.float64)
    fix = np.flatnonzero(gaps < GAP_THRESHOLD)
    for n in fix:
        xr = x[n].astype(np.float64)
        d2r = (x2[n].astype(np.float64)
               + c2
               - 2.0 * (cb64 @ xr))
        zr = g[n].astype(np.float64) - np.sqrt(d2r)
        k_new = int(np.argmax(zr))
        row = codes[n]
        nz = np.flatnonzero(row)
        if len(nz) == 1 and nz[0] == k_new:
            continue
        # undo whatever the device accumulated for this row, then redo
        cbsum -= row[:, None].astype(np.float64) * xr[None, :]
        counts -= row.astype(np.float64)
        row[:] = 0.0
        row[k_new] = 1.0
        cbsum[k_new] += xr
        counts[k_new] += 1.0

    # ---- assemble scalar outputs ----
    sum_x2 = float((x64 ** 2).sum())
    sum_d2_sel = sum_x2 + float(counts @ c2) - 2.0 * float((cbsum * cb64).sum())
    loss = np.float32((1.0 + COMMIT) * sum_d2_sel / (N * D))
    new_cb = (DECAY * cb64 + (1.0 - DECAY) * cbsum).astype(np.float32)
    return codes, loss, new_cb


# revision 19
# speedup vs baseline: 1.6353x; 1.6353x over previous
"""Trainium2 Bass kernel for DistanceGumbelSoftmaxVQ.

Reference reduces to (forward numerics):
  d2(n,k)  = ||x_n||^2 + ||c_k||^2 - 2 x_n.c_k
  z(n,k)   = gumbel(n,k) - sqrt(d2)          (gumbel: fixed key(42) tensor)
  idx_n    = argmax_k z
  codes    = one_hot(idx)                     (y_soft terms cancel exactly)
  loss     = 1.25 * mean_n d2(n, idx_n) / 1  ... = 1.25*mean((cb[idx]-x)^2)
  new_cb   = 0.99*cb + 0.01 * codes.T @ x

Sharding: data-parallel over N rows across 8 cores; codebook replicated.
Device per core: dist matmul (PE) -> sqrt w/ fused affine (ACT) ->
z = g - dist (DVE/GPSIMD split) -> row top-8 (DVE Max8) ->
one-hot via is_equal (GPSIMD) -> codes out; codes^T@x and counts
accumulated on PE into PSUM across all row tiles.

Host: gathers shards, sums partial cbsum/counts, assembles the loss from
counts/cbsum/c2/x2 (all idx-dependent terms), and exactly recomputes the
few rows whose top-2 gap is below a threshold (device sqrt is a ~1e-4-abs
spline approx; ambiguous rows are patched from f32-exact host values so
the argmax matches the reference bit-for-bit).
"""

import numpy as np

N, D, K = 65536, 128, 1024
NCORES = 8
R = N // NCORES            # rows per core
P = 128                    # partitions / rows per tile
SUB = 4                    # row-tiles per DMA chunk
DECAY = 0.99
COMMIT = 0.25
GAP_THRESHOLD = 5e-3       # host-correct rows with top2 gap below this

# fraction of the z = g - dist subtraction done on DVE (rest on GPSIMD)
ZSPLIT = 640

_CACHE = {}
TRACE = False          # set by test harness to collect a HW profile
LAST_RESULTS = None    # BassKernelResults of the last run (for profiling)


def _build_nc(rows):
    import concourse.bass as bass
    import concourse.mybir as mybir
    from concourse.tile import TileContext

    # --- walrus workaround: split tail-drain waits into single-wait NOPs ---
    from concourse.tile import TileContext as _TC
    from concourse.vector_clock import ScopedClock as _SC

    def _patched_drain(self, tick_clock, wait_clock):
        nc = self.nc
        probe = nc.sync.nop()
        wait_clock.add_sem_waits(probe.ins, _SC({None: tick_clock.global_clock}))
        si = probe.ins.sync_info
        waits = list(si.on_wait or []) if si is not None else []
        if si is not None:
            probe.ins.sync_info = mybir.SyncInfo(
                on_wait=waits[:1], on_update=list(si.on_update or [])
            )
        for w in waits[1:]:
            n2 = nc.sync.nop()
            n2.ins.sync_info = mybir.SyncInfo(on_wait=[w], on_update=[])
        nc.sync.drain()
        nc.all_engine_barrier()
        assert self.sems is not None
        popped = nc._tile_sem_poison_stack.pop()
        assert popped is self._sem_poison
        nc.clear_and_free_semaphores(list(self.sems.allocated().values()))
        nc.all_engine_barrier()

    _TC._drain_and_barrier = _patched_drain

    # --- walrus workaround #2: this compiler rejects instructions carrying
    # more than MAX_WAITS sync waits; hoist the excess onto same-engine NOPs
    # placed immediately before the instruction. ---
    MAX_WAITS = 1

    def _split_excess_waits(nc):
        n_split = 0
        for bbb in nc.bb_map.values():
            insts = bbb.bb.instructions
            out = []
            changed = False
            for inst in insts:
                si = getattr(inst, "sync_info", None)
                waits = list(si.on_wait or []) if si is not None else []
                if len(waits) > MAX_WAITS:
                    extra, keep = waits[:-MAX_WAITS], waits[-MAX_WAITS:]
                    for j in range(0, len(extra), MAX_WAITS):
                        nop = mybir.InstNoOp(
                            name=f"{inst.name}-wsplit-{j}",
                            sync_info=mybir.SyncInfo(
                                on_wait=extra[j:j + MAX_WAITS], on_update=[]
                            ),
                            engine=inst.engine,
                            bass_nofuse=True,
                        )
                        nc.register_instruction(nop, overwrite=True)
                        out.append(nop)
                        n_split += 1
                    inst.sync_info = mybir.SyncInfo(
                        on_wait=keep, on_update=list(si.on_update or [])
                    )
                    changed = True
                out.append(inst)
            if changed:
                insts.clear()
                insts.extend(out)
        return n_split

    f32 = mybir.dt.float32
    f32r = mybir.dt.float32r
    T = rows // P              # row tiles
    NCH = T // SUB             # DMA chunks

    nc = bass.Bass()
    x_d = nc.declare_dram_parameter("x", [rows, D], f32r, isOutput=False)
    xt_d = nc.declare_dram_parameter("xt", [D, rows], f32r, isOutput=False)
    x2_d = nc.declare_dram_parameter("x2", [P, T], f32, isOutput=False)
    g_d = nc.declare_dram_parameter("g", [rows, K], f32, isOutput=False)
    cbt_d = nc.declare_dram_parameter("cbt", [D, K], f32r, isOutput=False)
    c2h_d = nc.declare_dram_parameter("c2h", [1, K], f32r, isOutput=False)
    ones_d = nc.declare_dram_parameter("ones", [1, P], f32r, isOutput=False)
    codes_d = nc.declare_dram_parameter("codes", [rows, K], f32r, isOutput=True)
    cbsum_d = nc.declare_dram_parameter("cbsumT", [D, K], f32, isOutput=True)
    maxs_d = nc.declare_dram_parameter("maxs", [P, 2 * T], f32, isOutput=True)

    g_ap = g_d.rearrange("(c s p) k -> c p s k", p=P, s=SUB)
    codes_ap = codes_d.rearrange("(c s p) k -> c p s k", p=P, s=SUB)
    x_ap = x_d.rearrange("(t p) d -> p t d", p=P)

    with TileContext(nc) as tc:
        with (
            tc.tile_pool(name="persist", bufs=1) as pp,
            tc.tile_pool(name="gpool", bufs=2) as gp,
            tc.tile_pool(name="cpool", bufs=2) as cp,
            tc.tile_pool(name="work", bufs=3) as wp,
            tc.tile_pool(name="pd", bufs=2, space="PSUM") as pdp,
            tc.tile_pool(name="pacc", bufs=1, space="PSUM") as pap,
        ):
            # persistent loads
            x_sb = pp.tile([P, T, D], f32r)
            nc.sync.dma_start(out=x_sb[:], in_=x_ap[:])
            xt_sb = pp.tile([D, rows], f32r)
            nc.sync.dma_start(out=xt_sb[:], in_=xt_d[:])
            x2_sb = pp.tile([P, T], f32)
            nc.sync.dma_start(out=x2_sb[:], in_=x2_d[:])
            cbt_sb = pp.tile([D, K], f32r)
            nc.sync.dma_start(out=cbt_sb[:], in_=cbt_d[:])
            c2h_sb = pp.tile([1, K], f32r)
            nc.sync.dma_start(out=c2h_sb[:], in_=c2h_d[:])
            onesrow = pp.tile([1, P], f32r)
            nc.sync.dma_start(out=onesrow[:], in_=ones_d[:])
            maxs_sb = pp.tile([P, 2 * T], f32)

            pcbT = pap.tile([D, K], f32)        # x^T @ codes accumulator (cbsum^T)

            for c in range(NCH):
                g_sb = gp.tile([P, SUB, K], f32, tag="g")
                nc.sync.dma_start(out=g_sb[:], in_=g_ap[c])
                codes_sb = cp.tile([P, SUB, K], f32r, tag="codes")
                for s in range(SUB):
                    t = c * SUB + s
                    pd = pdp.tile([P, K], f32, tag="pd")
                    for h in range(2):
                        ksl = slice(h * 512, (h + 1) * 512)
                        nc.tensor.matmul(
                            pd[:, ksl],
                            lhsT=xt_sb[:, t * P:(t + 1) * P],
                            rhs=cbt_sb[:, ksl],
                            start=True, stop=False,
                        )
                        nc.tensor.matmul(
                            pd[:, ksl],
                            lhsT=onesrow[:1, :],
                            rhs=c2h_sb[:1, ksl],
                            start=False, stop=True,
                        )
                    # dist = sqrt(x2 - 2*pd)   (pd = x.c - 0.5*||c||^2)
                    dist = wp.tile([P, K], f32, tag="dist")
                    nc.scalar.activation(
                        dist[:], pd[:],
                        mybir.ActivationFunctionType.Sqrt,
                        bias=x2_sb[:, t:t + 1], scale=-2.0,
                    )
                    # z = g - dist  (split DVE / GPSIMD)
                    z = wp.tile([P, K], f32, tag="z")
                    gs = g_sb[:, s, :]
                    if ZSPLIT > 0:
                        nc.vector.tensor_sub(z[:, :ZSPLIT], gs[:, :ZSPLIT], dist[:, :ZSPLIT])
                    if ZSPLIT < K:
                        nc.gpsimd.tensor_sub(z[:, ZSPLIT:], gs[:, ZSPLIT:], dist[:, ZSPLIT:])
                    # top-8 per row
                    m8 = wp.tile([P, 8], f32, tag="m8")
                    nc.vector.max(out=m8[:], in_=z[:])
                    nc.scalar.copy(out=maxs_sb[:, 2 * t:2 * t + 2], in_=m8[:, 0:2])
                    # one-hot: codes = (z == rowmax)
                    nc.gpsimd.tensor_scalar(
                        codes_sb[:, s, :], z[:], m8[:, 0:1], None,
                        op0=mybir.AluOpType.is_equal,
                    )
                    # accumulate x^T @ codes
                    for h in range(2):
                        ksl = slice(h * 512, (h + 1) * 512)
                        nc.tensor.matmul(
                            pcbT[:, ksl], lhsT=x_sb[:, t, :],
                            rhs=codes_sb[:, s, ksl],
                            start=(t == 0), stop=(t == T - 1),
                        )
                nc.sync.dma_start(out=codes_ap[c], in_=codes_sb[:])

            cbs_sb = pp.tile([D, K], f32)
            nc.scalar.copy(out=cbs_sb[:], in_=pcbT[:])
            nc.sync.dma_start(out=cbsum_d[:], in_=cbs_sb[:])
            nc.sync.dma_start(out=maxs_d[:], in_=maxs_sb[:])

    _split_excess_waits(nc)
    return nc


def _host_prep(inputs_np, codebook_np):
    """Host-side constant/derived tensors. Gumbel must match
    jax.random.gumbel(key(42), (N,K), float32) bit-for-bit."""
    if "g" not in _CACHE:
        import jax
        import jax.numpy as jnp
        g = np.asarray(jax.random.gumbel(jax.random.key(42), (N, K), jnp.float32))
        _CACHE["g"] = g
    g = _CACHE["g"]
    x = np.ascontiguousarray(inputs_np, dtype=np.float32)
    cb = np.ascontiguousarray(codebook_np, dtype=np.float32)
    cbt = np.ascontiguousarray(cb.T)
    c2 = (cb.astype(np.float64) ** 2).sum(1)
    c2h = np.ascontiguousarray((-0.5 * c2).astype(np.float32)[None, :])
    x2 = (x.astype(np.float64) ** 2).sum(1).astype(np.float32)
    return g, x, cb, cbt, c2, c2h, x2


def build_inmaps(x, g, cbt, c2h, x2):
    T = R // P
    in_maps = []
    for i in range(NCORES):
        rs = slice(i * R, (i + 1) * R)
        xs = x[rs]
        in_maps.append({
            "x": xs,
            "xt": np.ascontiguousarray(xs.T),
            "x2": np.ascontiguousarray(x2[rs].reshape(T, P).T),
            "g": g[rs],
            "cbt": cbt,
            "c2h": c2h,
            "ones": np.ones((1, P), dtype=np.float32),
        })
    return in_maps


def get_nc():
    if "nc" not in _CACHE:
        _CACHE["nc"] = _build_nc(R)
    return _CACHE["nc"]


def kernel(inputs, codebook):
    from concourse.bass_utils import run_bass_kernel_spmd

    g, x, cb, cbt, c2, c2h, x2 = _host_prep(np.asarray(inputs), np.asarray(codebook))
    T = R // P
    nc = get_nc()
    in_maps = build_inmaps(x, g, cbt, c2h, x2)

    res = run_bass_kernel_spmd(nc, in_maps, list(range(NCORES)), trace=TRACE)
    global LAST_RESULTS
    LAST_RESULTS = res

    codes = np.empty((N, K), dtype=np.float32)
    cbsum = np.zeros((K, D), dtype=np.float64)
    gaps = np.empty(N, dtype=np.float32)
    for i in range(NCORES):
        r = res.results[i]
        rs = slice(i * R, (i + 1) * R)
        codes[rs] = r["codes"]
        cbsum += r["cbsumT"].T.astype(np.float64)
        m = r["maxs"].reshape(P, T, 2)
        gaps[rs] = (m[:, :, 0] - m[:, :, 1]).T.reshape(R)

    # ---- exact host correction of near-tie rows ----
    x64 = x.astype(np.float64)
    cb64 = cb.astype(np.float64)
    fix = np.flatnonzero(gaps < GAP_THRESHOLD)
    for n in fix:
        xr = x[n].astype(np.float64)
        d2r = (x2[n].astype(np.float64)
               + c2
               - 2.0 * (cb64 @ xr))
        zr = g[n].astype(np.float64) - np.sqrt(d2r)
        k_new = int(np.argmax(zr))
        row = codes[n]
        nz = np.flatnonzero(row)
        if len(nz) == 1 and nz[0] == k_new:
            continue
        # undo whatever the device accumulated for this row, then redo
        cbsum -= row[:, None].astype(np.float64) * xr[None, :]
        row[:] = 0.0
        row[k_new] = 1.0
        cbsum[k_new] += xr

    # ---- assemble scalar outputs ----
    counts = codes.sum(0, dtype=np.float64)
    sum_x2 = float((x64 ** 2).sum())
    sum_d2_sel = sum_x2 + float(counts @ c2) - 2.0 * float((cbsum * cb64).sum())
    loss = np.float32((1.0 + COMMIT) * sum_d2_sel / (N * D))
    new_cb = (DECAY * cb64 + (1.0 - DECAY) * cbsum).astype(np.float32)
    return codes, loss, new_cb


# revision 20
# speedup vs baseline: 2.0697x; 1.2657x over previous
"""Trainium2 Bass kernel for DistanceGumbelSoftmaxVQ.

Reference reduces to (forward numerics):
  d2(n,k)  = ||x_n||^2 + ||c_k||^2 - 2 x_n.c_k
  z(n,k)   = gumbel(n,k) - sqrt(d2)          (gumbel: fixed key(42) tensor)
  idx_n    = argmax_k z
  codes    = one_hot(idx)                     (y_soft terms cancel exactly)
  loss     = 1.25 * mean_n d2(n, idx_n) / 1  ... = 1.25*mean((cb[idx]-x)^2)
  new_cb   = 0.99*cb + 0.01 * codes.T @ x

Sharding: data-parallel over N rows across 8 cores; codebook replicated.
Device per core: dist matmul (PE) -> sqrt w/ fused affine (ACT) ->
z = g - dist (DVE/GPSIMD split) -> row top-8 (DVE Max8) ->
one-hot via is_equal (GPSIMD) -> codes out; codes^T@x and counts
accumulated on PE into PSUM across all row tiles.

Host: gathers shards, sums partial cbsum/counts, assembles the loss from
counts/cbsum/c2/x2 (all idx-dependent terms), and exactly recomputes the
few rows whose top-2 gap is below a threshold (device sqrt is a ~1e-4-abs
spline approx; ambiguous rows are patched from f32-exact host values so
the argmax matches the reference bit-for-bit).
"""

import numpy as np

N, D, K = 65536, 128, 1024
NCORES = 8
R = N // NCORES            # rows per core
P = 128                    # partitions / rows per tile
SUB = 8                    # row-tiles per DMA chunk
DECAY = 0.99
COMMIT = 0.25
GAP_THRESHOLD = 8e-3       # host-correct rows with top2 gap below this
GSCALE = 1024.0            # gumbel fixed-point scale (int16 = g * GSCALE)

# fraction of the z = g - dist subtraction done on DVE (rest on GPSIMD)
ZSPLIT = 640

_CACHE = {}
TRACE = False          # set by test harness to collect a HW profile
LAST_RESULTS = None    # BassKernelResults of the last run (for profiling)


def _build_nc(rows):
    import concourse.bass as bass
    import concourse.mybir as mybir
    from concourse.tile import TileContext

    # --- walrus workaround: split tail-drain waits into single-wait NOPs ---
    from concourse.tile import TileContext as _TC
    from concourse.vector_clock import ScopedClock as _SC

    def _patched_drain(self, tick_clock, wait_clock):
        nc = self.nc
        probe = nc.sync.nop()
        wait_clock.add_sem_waits(probe.ins, _SC({None: tick_clock.global_clock}))
        si = probe.ins.sync_info
        waits = list(si.on_wait or []) if si is not None else []
        if si is not None:
            probe.ins.sync_info = mybir.SyncInfo(
                on_wait=waits[:1], on_update=list(si.on_update or [])
            )
        for w in waits[1:]:
            n2 = nc.sync.nop()
            n2.ins.sync_info = mybir.SyncInfo(on_wait=[w], on_update=[])
        nc.sync.drain()
        nc.all_engine_barrier()
        assert self.sems is not None
        popped = nc._tile_sem_poison_stack.pop()
        assert popped is self._sem_poison
        nc.clear_and_free_semaphores(list(self.sems.allocated().values()))
        nc.all_engine_barrier()

    _TC._drain_and_barrier = _patched_drain

    # --- walrus workaround #2: this compiler rejects instructions carrying
    # more than MAX_WAITS sync waits; hoist the excess onto same-engine NOPs
    # placed immediately before the instruction. ---
    MAX_WAITS = 1

    def _split_excess_waits(nc):
        n_split = 0
        for bbb in nc.bb_map.values():
            insts = bbb.bb.instructions
            out = []
            changed = False
            for inst in insts:
                si = getattr(inst, "sync_info", None)
                waits = list(si.on_wait or []) if si is not None else []
                if len(waits) > MAX_WAITS:
                    extra, keep = waits[:-MAX_WAITS], waits[-MAX_WAITS:]
                    for j in range(0, len(extra), MAX_WAITS):
                        nop = mybir.InstNoOp(
                            name=f"{inst.name}-wsplit-{j}",
                            sync_info=mybir.SyncInfo(
                                on_wait=extra[j:j + MAX_WAITS], on_update=[]
                            ),
                            engine=inst.engine,
                            bass_nofuse=True,
                        )
                        nc.register_instruction(nop, overwrite=True)
                        out.append(nop)
                        n_split += 1
                    inst.sync_info = mybir.SyncInfo(
                        on_wait=keep, on_update=list(si.on_update or [])
                    )
                    changed = True
                out.append(inst)
            if changed:
                insts.clear()
                insts.extend(out)
        return n_split

    f32 = mybir.dt.float32
    f32r = mybir.dt.float32r
    bf16 = mybir.dt.bfloat16
    i16 = mybir.dt.int16
    T = rows // P              # row tiles
    NCH = T // SUB             # DMA chunks

    nc = bass.Bass()
    x_d = nc.declare_dram_parameter("x", [rows, D], bf16, isOutput=False)
    xt_d = nc.declare_dram_parameter("xt", [D, rows], f32r, isOutput=False)
    x2_d = nc.declare_dram_parameter("x2", [P, T], f32, isOutput=False)
    g_d = nc.declare_dram_parameter("g", [rows, K], i16, isOutput=False)
    cbt_d = nc.declare_dram_parameter("cbt", [D, K], f32r, isOutput=False)
    c2h_d = nc.declare_dram_parameter("c2h", [1, K], f32r, isOutput=False)
    ones_d = nc.declare_dram_parameter("ones", [1, P], f32r, isOutput=False)
    codes_d = nc.declare_dram_parameter("codes", [rows, K], bf16, isOutput=True)
    cbsum_d = nc.declare_dram_parameter("cbsumT", [D, K], f32, isOutput=True)
    maxs_d = nc.declare_dram_parameter("maxs", [P, 2 * T], f32, isOutput=True)

    g_ap = g_d.rearrange("(c s p) k -> c p s k", p=P, s=SUB)
    codes_ap = codes_d.rearrange("(c s p) k -> c p s k", p=P, s=SUB)
    x_ap = x_d.rearrange("(t p) d -> p t d", p=P)

    with TileContext(nc) as tc:
        with (
            tc.tile_pool(name="persist", bufs=1) as pp,
            tc.tile_pool(name="gpool", bufs=2) as gp,
            tc.tile_pool(name="cpool", bufs=2) as cp,
            tc.tile_pool(name="work", bufs=3) as wp,
            tc.tile_pool(name="pd", bufs=2, space="PSUM") as pdp,
            tc.tile_pool(name="pacc", bufs=1, space="PSUM") as pap,
        ):
            # persistent loads
            x_sb = pp.tile([P, T, D], bf16)
            nc.sync.dma_start(out=x_sb[:], in_=x_ap[:])
            xt_sb = pp.tile([D, rows], f32r)
            nc.sync.dma_start(out=xt_sb[:], in_=xt_d[:])
            x2_sb = pp.tile([P, T], f32)
            nc.sync.dma_start(out=x2_sb[:], in_=x2_d[:])
            cbt_sb = pp.tile([D, K], f32r)
            nc.sync.dma_start(out=cbt_sb[:], in_=cbt_d[:])
            c2h_sb = pp.tile([1, K], f32r)
            nc.sync.dma_start(out=c2h_sb[:], in_=c2h_d[:])
            onesrow = pp.tile([1, P], f32r)
            nc.sync.dma_start(out=onesrow[:], in_=ones_d[:])
            maxs_sb = pp.tile([P, 2 * T], f32)

            pcbT = pap.tile([D, K], f32)        # x^T @ codes accumulator (cbsum^T)

            for c in range(NCH):
                g_sb = gp.tile([P, SUB, K], i16, tag="g")
                nc.sync.dma_start(out=g_sb[:], in_=g_ap[c])
                codes_sb = cp.tile([P, SUB, K], bf16, tag="codes")
                for s in range(SUB):
                    t = c * SUB + s
                    pd = pdp.tile([P, K], f32, tag="pd")
                    for h in range(2):
                        ksl = slice(h * 512, (h + 1) * 512)
                        nc.tensor.matmul(
                            pd[:, ksl],
                            lhsT=xt_sb[:, t * P:(t + 1) * P],
                            rhs=cbt_sb[:, ksl],
                            start=True, stop=False,
                        )
                        nc.tensor.matmul(
                            pd[:, ksl],
                            lhsT=onesrow[:1, :],
                            rhs=c2h_sb[:1, ksl],
                            start=False, stop=True,
                        )
                    # dist = sqrt(x2 - 2*pd)   (pd = x.c - 0.5*||c||^2)
                    dist = wp.tile([P, K], f32, tag="dist")
                    nc.scalar.activation(
                        dist[:], pd[:],
                        mybir.ActivationFunctionType.Sqrt,
                        bias=x2_sb[:, t:t + 1], scale=-2.0 * GSCALE * GSCALE,
                    )
                    # z = g - dist  (split DVE / GPSIMD)
                    z = wp.tile([P, K], f32, tag="z")
                    gs = g_sb[:, s, :]
                    if ZSPLIT > 0:
                        nc.vector.tensor_sub(z[:, :ZSPLIT], gs[:, :ZSPLIT], dist[:, :ZSPLIT])
                    if ZSPLIT < K:
                        nc.gpsimd.tensor_sub(z[:, ZSPLIT:], gs[:, ZSPLIT:], dist[:, ZSPLIT:])
                    # top-8 per row
                    m8 = wp.tile([P, 8], f32, tag="m8")
                    nc.vector.max(out=m8[:], in_=z[:])
                    nc.scalar.copy(out=maxs_sb[:, 2 * t:2 * t + 2], in_=m8[:, 0:2])
                    # one-hot: codes = (z == rowmax)
                    nc.vector.tensor_scalar(
                        codes_sb[:, s, :], z[:], m8[:, 0:1], None,
                        op0=mybir.AluOpType.is_equal,
                    )
                    # accumulate x^T @ codes
                    for h in range(2):
                        ksl = slice(h * 512, (h + 1) * 512)
                        nc.tensor.matmul(
                            pcbT[:, ksl], lhsT=x_sb[:, t, :],
                            rhs=codes_sb[:, s, ksl],
                            start=(t == 0), stop=(t == T - 1),
                        )
                nc.sync.dma_start(out=codes_ap[c], in_=codes_sb[:])

            cbs_sb = pp.tile([D, K], f32)
            nc.scalar.copy(out=cbs_sb[:], in_=pcbT[:])
            nc.sync.dma_start(out=cbsum_d[:], in_=cbs_sb[:])
            nc.sync.dma_start(out=maxs_d[:], in_=maxs_sb[:])

    _split_excess_waits(nc)
    return nc


def _host_prep(inputs_np, codebook_np):
    """Host-side constant/derived tensors. Gumbel must match
    jax.random.gumbel(key(42), (N,K), float32) bit-for-bit."""
    if "g" not in _CACHE:
        import jax
        import jax.numpy as jnp
        g = np.asarray(jax.random.gumbel(jax.random.key(42), (N, K), jnp.float32))
        _CACHE["g"] = g
    g = _CACHE["g"]
    x = np.ascontiguousarray(inputs_np, dtype=np.float32)
    cb = np.ascontiguousarray(codebook_np, dtype=np.float32)
    cbt = np.ascontiguousarray(cb.T)
    c2 = (cb.astype(np.float64) ** 2).sum(1)
    c2h = np.ascontiguousarray((-0.5 * c2).astype(np.float32)[None, :])
    x2 = (x.astype(np.float64) ** 2).sum(1).astype(np.float32)
    return g, x, cb, cbt, c2, c2h, x2


def build_inmaps(x, g, cbt, c2h, x2):
    import ml_dtypes
    T = R // P
    g_i16 = _CACHE.get("g_i16")
    if g_i16 is None:
        g_i16 = np.clip(np.rint(g * GSCALE), -32767, 32767).astype(np.int16)
        _CACHE["g_i16"] = g_i16
    x_bf = x.astype(ml_dtypes.bfloat16)
    x2s = (x2.astype(np.float64) * GSCALE * GSCALE).astype(np.float32)
    in_maps = []
    for i in range(NCORES):
        rs = slice(i * R, (i + 1) * R)
        in_maps.append({
            "x": x_bf[rs],
            "xt": np.ascontiguousarray(x[rs].T),
            "x2": np.ascontiguousarray(x2s[rs].reshape(T, P).T),
            "g": g_i16[rs],
            "cbt": cbt,
            "c2h": c2h,
            "ones": np.ones((1, P), dtype=np.float32),
        })
    return in_maps


def get_nc():
    if "nc" not in _CACHE:
        _CACHE["nc"] = _build_nc(R)
    return _CACHE["nc"]


def kernel(inputs, codebook):
    from concourse.bass_utils import run_bass_kernel_spmd

    g, x, cb, cbt, c2, c2h, x2 = _host_prep(np.asarray(inputs), np.asarray(codebook))
    T = R // P
    nc = get_nc()
    in_maps = build_inmaps(x, g, cbt, c2h, x2)

    res = run_bass_kernel_spmd(nc, in_maps, list(range(NCORES)), trace=TRACE)
    global LAST_RESULTS
    LAST_RESULTS = res

    codes = np.empty((N, K), dtype=np.float32)
    cbsum = np.zeros((K, D), dtype=np.float64)
    gaps = np.empty(N, dtype=np.float32)
    for i in range(NCORES):
        r = res.results[i]
        rs = slice(i * R, (i + 1) * R)
        codes[rs] = r["codes"].astype(np.float32)
        cbsum += r["cbsumT"].T.astype(np.float64)
        m = r["maxs"].reshape(P, T, 2)
        gaps[rs] = (m[:, :, 0] - m[:, :, 1]).T.reshape(R) / np.float32(GSCALE)

    # ---- exact host correction of near-tie rows ----
    x64 = x.astype(np.float64)
    cb64 = cb.astype(np.float64)
    fix = np.flatnonzero(gaps < GAP_THRESHOLD)
    for n in fix:
        xr = x[n].astype(np.float64)
        d2r = (x2[n].astype(np.float64)
               + c2
               - 2.0 * (cb64 @ xr))
        zr = g[n].astype(np.float64) - np.sqrt(d2r)
        k_new = int(np.argmax(zr))
        row = codes[n]
        nz = np.flatnonzero(row)
        if len(nz) == 1 and nz[0] == k_new:
            continue
        # undo whatever the device accumulated for this row, then redo
        cbsum -= row[:, None].astype(np.float64) * xr[None, :]
        row[:] = 0.0
        row[k_new] = 1.0
        cbsum[k_new] += xr

    # ---- assemble scalar outputs ----
    counts = codes.sum(0, dtype=np.float64)
    sum_x2 = float((x64 ** 2).sum())
    sum_d2_sel = sum_x2 + float(counts @ c2) - 2.0 * float((cbsum * cb64).sum())
    loss = np.float32((1.0 + COMMIT) * sum_d2_sel / (N * D))
    new_cb = (DECAY * cb64 + (1.0 - DECAY) * cbsum).astype(np.float32)
    return codes, loss, new_cb


# revision 22
# speedup vs baseline: 6.7547x; 3.2636x over previous
"""Trainium2 Bass kernel for DistanceGumbelSoftmaxVQ.

Reference reduces to (forward numerics):
  d2(n,k)  = ||x_n||^2 + ||c_k||^2 - 2 x_n.c_k
  z(n,k)   = gumbel(n,k) - sqrt(d2)          (gumbel: fixed key(42) tensor)
  idx_n    = argmax_k z
  codes    = one_hot(idx)                     (y_soft terms cancel exactly)
  loss     = 1.25 * mean_n d2(n, idx_n) / 1  ... = 1.25*mean((cb[idx]-x)^2)
  new_cb   = 0.99*cb + 0.01 * codes.T @ x

Sharding: data-parallel over N rows across 8 cores; codebook replicated.
Device per core: dist matmul (PE) -> sqrt w/ fused affine (ACT) ->
z = g - dist (DVE/GPSIMD split) -> row top-8 (DVE Max8) ->
one-hot via is_equal (GPSIMD) -> codes out; codes^T@x and counts
accumulated on PE into PSUM across all row tiles.

Host: gathers shards, sums partial cbsum/counts, assembles the loss from
counts/cbsum/c2/x2 (all idx-dependent terms), and exactly recomputes the
few rows whose top-2 gap is below a threshold (device sqrt is a ~1e-4-abs
spline approx; ambiguous rows are patched from f32-exact host values so
the argmax matches the reference bit-for-bit).
"""

import numpy as np

N, D, K = 65536, 128, 1024
NCORES = 8
R = N // NCORES            # rows per core
P = 128                    # partitions / rows per tile
SUB = 8                    # row-tiles per DMA chunk
DECAY = 0.99
COMMIT = 0.25
GAP_THRESHOLD = 8e-3       # host-correct rows with top2 gap below this
GSCALE = 1024.0            # gumbel fixed-point scale (int16 = g * GSCALE)

# leading columns of the z = g - dist subtraction done on DVE (rest on GPSIMD)
ZSPLIT = 0

_CACHE = {}
TRACE = False          # set by test harness to collect a HW profile
VERBOSE = False        # print correction statistics
LAST_RESULTS = None    # BassKernelResults of the last run (for profiling)


def _build_nc(rows, loops=1):
    import concourse.bass as bass
    import concourse.mybir as mybir
    from concourse.tile import TileContext

    # --- walrus workaround: split tail-drain waits into single-wait NOPs ---
    from concourse.tile import TileContext as _TC
    from concourse.vector_clock import ScopedClock as _SC

    def _patched_drain(self, tick_clock, wait_clock):
        nc = self.nc
        probe = nc.sync.nop()
        wait_clock.add_sem_waits(probe.ins, _SC({None: tick_clock.global_clock}))
        si = probe.ins.sync_info
        waits = list(si.on_wait or []) if si is not None else []
        if si is not None:
            probe.ins.sync_info = mybir.SyncInfo(
                on_wait=waits[:1], on_update=list(si.on_update or [])
            )
        for w in waits[1:]:
            n2 = nc.sync.nop()
            n2.ins.sync_info = mybir.SyncInfo(on_wait=[w], on_update=[])
        nc.sync.drain()
        nc.all_engine_barrier()
        assert self.sems is not None
        popped = nc._tile_sem_poison_stack.pop()
        assert popped is self._sem_poison
        nc.clear_and_free_semaphores(list(self.sems.allocated().values()))
        nc.all_engine_barrier()

    _TC._drain_and_barrier = _patched_drain

    # --- walrus workaround #2: this compiler rejects instructions carrying
    # more than MAX_WAITS sync waits; hoist the excess onto same-engine NOPs
    # placed immediately before the instruction. ---
    MAX_WAITS = 1

    def _split_excess_waits(nc):
        n_split = 0
        for bbb in nc.bb_map.values():
            insts = bbb.bb.instructions
            out = []
            changed = False
            for inst in insts:
                si = getattr(inst, "sync_info", None)
                waits = list(si.on_wait or []) if si is not None else []
                if len(waits) > MAX_WAITS:
                    extra, keep = waits[:-MAX_WAITS], waits[-MAX_WAITS:]
                    for j in range(0, len(extra), MAX_WAITS):
                        nop = mybir.InstNoOp(
                            name=f"{inst.name}-wsplit-{j}",
                            sync_info=mybir.SyncInfo(
                                on_wait=extra[j:j + MAX_WAITS], on_update=[]
                            ),
                            engine=inst.engine,
                            bass_nofuse=True,
                        )
                        nc.register_instruction(nop, overwrite=True)
                        out.append(nop)
                        n_split += 1
                    inst.sync_info = mybir.SyncInfo(
                        on_wait=keep, on_update=list(si.on_update or [])
                    )
                    changed = True
                out.append(inst)
            if changed:
                insts.clear()
                insts.extend(out)
        return n_split

    f32 = mybir.dt.float32
    f32r = mybir.dt.float32r
    bf16 = mybir.dt.bfloat16
    i16 = mybir.dt.int16
    T = rows // P              # row tiles
    NCH = T // SUB             # DMA chunks

    nc = bass.Bass()
    x_d = nc.declare_dram_parameter("x", [P, T, D], bf16, isOutput=False)
    xt_d = nc.declare_dram_parameter("xt", [D, rows], f32r, isOutput=False)
    x2_d = nc.declare_dram_parameter("x2", [P, T], f32, isOutput=False)
    g_d = nc.declare_dram_parameter("g", [P, NCH, SUB, K], i16, isOutput=False)
    cbt_d = nc.declare_dram_parameter("cbt", [D, K], f32r, isOutput=False)
    c2h_d = nc.declare_dram_parameter("c2h", [1, K], f32r, isOutput=False)
    ones_d = nc.declare_dram_parameter("ones", [1, P], f32r, isOutput=False)
    codes_d = nc.declare_dram_parameter("codes", [P, NCH, SUB, K], bf16, isOutput=True)
    cbsum_d = nc.declare_dram_parameter("cbsumT", [D, K], f32, isOutput=True)
    maxs_d = nc.declare_dram_parameter("maxs", [P, 2 * T], f32, isOutput=True)


    with TileContext(nc) as tc:
        with (
            tc.tile_pool(name="persist", bufs=1) as pp,
            tc.tile_pool(name="gpool", bufs=2) as gp,
            tc.tile_pool(name="cpool", bufs=2) as cp,
            tc.tile_pool(name="work", bufs=3) as wp,
            tc.tile_pool(name="pd", bufs=2, space="PSUM") as pdp,
            tc.tile_pool(name="pacc", bufs=1, space="PSUM") as pap,
        ):
            # persistent loads
            x_sb = pp.tile([P, T, D], bf16)
            nc.sync.dma_start(out=x_sb[:], in_=x_d[:])
            xt_sb = pp.tile([D, rows], f32r)
            nc.sync.dma_start(out=xt_sb[:], in_=xt_d[:])
            x2_sb = pp.tile([P, T], f32)
            nc.sync.dma_start(out=x2_sb[:], in_=x2_d[:])
            cbt_sb = pp.tile([D, K], f32r)
            nc.sync.dma_start(out=cbt_sb[:], in_=cbt_d[:])
            c2h_sb = pp.tile([1, K], f32r)
            nc.sync.dma_start(out=c2h_sb[:], in_=c2h_d[:])
            onesrow = pp.tile([1, P], f32r)
            nc.sync.dma_start(out=onesrow[:], in_=ones_d[:])
            maxs_sb = pp.tile([P, 2 * T], f32)

            pcbT = pap.tile([D, K], f32)        # x^T @ codes accumulator (cbsum^T)

            for it in range(loops):
              for c in range(NCH):
                g_sb = gp.tile([P, SUB, K], i16, tag="g")
                nc.sync.dma_start(out=g_sb[:], in_=g_d[:, c, :, :])
                codes_sb = cp.tile([P, SUB, K], bf16, tag="codes")
                for s in range(SUB):
                    t = c * SUB + s
                    pd = pdp.tile([P, K], f32, tag="pd")
                    for h in range(2):
                        ksl = slice(h * 512, (h + 1) * 512)
                        nc.tensor.matmul(
                            pd[:, ksl],
                            lhsT=xt_sb[:, t * P:(t + 1) * P],
                            rhs=cbt_sb[:, ksl],
                            start=True, stop=False,
                        )
                        nc.tensor.matmul(
                            pd[:, ksl],
                            lhsT=onesrow[:1, :],
                            rhs=c2h_sb[:1, ksl],
                            start=False, stop=True,
                        )
                    # dist = sqrt(x2 - 2*pd)   (pd = x.c - 0.5*||c||^2)
                    dist = wp.tile([P, K], f32, tag="dist")
                    nc.scalar.activation(
                        dist[:], pd[:],
                        mybir.ActivationFunctionType.Sqrt,
                        bias=x2_sb[:, t:t + 1], scale=-2.0 * GSCALE * GSCALE,
                    )
                    # z = g - dist  (split DVE / GPSIMD)
                    z = wp.tile([P, K], f32, tag="z")
                    gs = g_sb[:, s, :]
                    if ZSPLIT > 0:
                        nc.vector.tensor_sub(z[:, :ZSPLIT], gs[:, :ZSPLIT], dist[:, :ZSPLIT])
                    if ZSPLIT < K:
                        nc.gpsimd.tensor_sub(z[:, ZSPLIT:], gs[:, ZSPLIT:], dist[:, ZSPLIT:])
                    # top-8 per row
                    m8 = wp.tile([P, 8], f32, tag="m8")
                    nc.vector.max(out=m8[:], in_=z[:])
                    nc.scalar.copy(out=maxs_sb[:, 2 * t:2 * t + 2], in_=m8[:, 0:2])
                    # one-hot: codes = (z == rowmax)
                    nc.vector.tensor_scalar(
                        codes_sb[:, s, :], z[:], m8[:, 0:1], None,
                        op0=mybir.AluOpType.is_equal,
                    )
                    # accumulate x^T @ codes
                    for h in range(2):
                        ksl = slice(h * 512, (h + 1) * 512)
                        nc.tensor.matmul(
                            pcbT[:, ksl], lhsT=x_sb[:, t, :],
                            rhs=codes_sb[:, s, ksl],
                            start=(t == 0), stop=(t == T - 1),
                        )
                nc.sync.dma_start(out=codes_d[:, c, :, :], in_=codes_sb[:])

            cbs_sb = pp.tile([D, K], f32)
            nc.scalar.copy(out=cbs_sb[:], in_=pcbT[:])
            nc.sync.dma_start(out=cbsum_d[:], in_=cbs_sb[:])
            nc.sync.dma_start(out=maxs_d[:], in_=maxs_sb[:])

    _split_excess_waits(nc)
    return nc


def _host_prep(inputs_np, codebook_np):
    """Host-side constant/derived tensors. Gumbel must match
    jax.random.gumbel(key(42), (N,K), float32) bit-for-bit."""
    if "g" not in _CACHE:
        import jax
        import jax.numpy as jnp
        g = np.asarray(jax.random.gumbel(jax.random.key(42), (N, K), jnp.float32))
        _CACHE["g"] = g
    g = _CACHE["g"]
    x = np.ascontiguousarray(inputs_np, dtype=np.float32)
    cb = np.ascontiguousarray(codebook_np, dtype=np.float32)
    cbt = np.ascontiguousarray(cb.T)
    c2 = (cb.astype(np.float64) ** 2).sum(1)
    c2h = np.ascontiguousarray((-0.5 * c2).astype(np.float32)[None, :])
    x2 = (x.astype(np.float64) ** 2).sum(1).astype(np.float32)
    return g, x, cb, cbt, c2, c2h, x2


def build_inmaps(x, g, cbt, c2h, x2):
    import ml_dtypes
    T = R // P
    NCH = T // SUB
    g_re = _CACHE.get("g_re")
    if g_re is None:
        g_i16 = np.clip(np.rint(g * GSCALE), -32767, 32767).astype(np.int16)
        # (N,K) -> per core (P, NCH, SUB, K) with row n = c*SUB*P + s*P + p
        g_re = np.ascontiguousarray(
            g_i16.reshape(NCORES, NCH, SUB, P, K).transpose(0, 3, 1, 2, 4))
        _CACHE["g_re"] = g_re
    x_re = np.ascontiguousarray(
        x.reshape(NCORES, T, P, D).transpose(0, 2, 1, 3)).astype(ml_dtypes.bfloat16)
    x2s = (x2.astype(np.float64) * GSCALE * GSCALE).astype(np.float32)
    in_maps = []
    for i in range(NCORES):
        rs = slice(i * R, (i + 1) * R)
        in_maps.append({
            "x": x_re[i],
            "xt": np.ascontiguousarray(x[rs].T),
            "x2": np.ascontiguousarray(x2s[rs].reshape(T, P).T),
            "g": g_re[i],
            "cbt": cbt,
            "c2h": c2h,
            "ones": np.ones((1, P), dtype=np.float32),
        })
    return in_maps


def get_nc():
    if "nc" not in _CACHE:
        _CACHE["nc"] = _build_nc(R)
    return _CACHE["nc"]


def kernel(inputs, codebook):
    from concourse.bass_utils import run_bass_kernel_spmd

    g, x, cb, cbt, c2, c2h, x2 = _host_prep(np.asarray(inputs), np.asarray(codebook))
    T = R // P
    nc = get_nc()
    in_maps = build_inmaps(x, g, cbt, c2h, x2)

    res = run_bass_kernel_spmd(nc, in_maps, list(range(NCORES)), trace=TRACE)
    global LAST_RESULTS
    LAST_RESULTS = res

    codes = np.empty((N, K), dtype=np.float32)
    cbsum = np.zeros((K, D), dtype=np.float64)
    gaps = np.empty(N, dtype=np.float32)
    for i in range(NCORES):
        r = res.results[i]
        rs = slice(i * R, (i + 1) * R)
        codes[rs] = r["codes"].transpose(1, 2, 0, 3).reshape(R, K).astype(np.float32)
        cbsum += r["cbsumT"].T.astype(np.float64)
        m = r["maxs"].reshape(P, T, 2)
        gaps[rs] = (m[:, :, 0] - m[:, :, 1]).T.reshape(R) / np.float32(GSCALE)

    # ---- exact host correction of near-tie rows ----
    x64 = x.astype(np.float64)
    cb64 = cb.astype(np.float64)
    fix = np.flatnonzero(gaps < GAP_THRESHOLD)
    n_flip = 0
    for n in fix:
        xr = x[n].astype(np.float64)
        d2r = (x2[n].astype(np.float64)
               + c2
               - 2.0 * (cb64 @ xr))
        zr = g[n].astype(np.float64) - np.sqrt(d2r)
        k_new = int(np.argmax(zr))
        row = codes[n]
        nz = np.flatnonzero(row)
        if len(nz) == 1 and nz[0] == k_new:
            continue
        n_flip += 1
        # undo whatever the device accumulated for this row, then redo
        cbsum -= row[:, None].astype(np.float64) * xr[None, :]
        row[:] = 0.0
        row[k_new] = 1.0
        cbsum[k_new] += xr

    if VERBOSE:
        print(f"[kernel] corrected rows checked: {len(fix)}, flipped: {n_flip}")

    # ---- assemble scalar outputs ----
    counts = codes.sum(0, dtype=np.float64)
    sum_x2 = float((x64 ** 2).sum())
    sum_d2_sel = sum_x2 + float(counts @ c2) - 2.0 * float((cbsum * cb64).sum())
    loss = np.float32((1.0 + COMMIT) * sum_d2_sel / (N * D))
    new_cb = (DECAY * cb64 + (1.0 - DECAY) * cbsum).astype(np.float32)
    return codes, loss, new_cb


# revision 27
# speedup vs baseline: 11.3637x; 1.6823x over previous
"""Trainium2 Bass kernel for DistanceGumbelSoftmaxVQ.

Reference reduces to (forward numerics):
  d2(n,k)  = ||x_n||^2 + ||c_k||^2 - 2 x_n.c_k
  z(n,k)   = gumbel(n,k) - sqrt(d2)          (gumbel: fixed key(42) tensor)
  idx_n    = argmax_k z
  codes    = one_hot(idx)                     (y_soft terms cancel exactly)
  loss     = 1.25 * mean_n d2(n, idx_n) / 1  ... = 1.25*mean((cb[idx]-x)^2)
  new_cb   = 0.99*cb + 0.01 * codes.T @ x

Sharding: data-parallel over N rows across 8 cores; codebook replicated.
Device per core: dist matmul (PE) -> sqrt w/ fused affine (ACT) ->
z = g - dist (DVE/GPSIMD split) -> row top-8 (DVE Max8) ->
one-hot via is_equal (GPSIMD) -> codes out; codes^T@x and counts
accumulated on PE into PSUM across all row tiles.

Host: gathers shards, sums partial cbsum/counts, assembles the loss from
counts/cbsum/c2/x2 (all idx-dependent terms), and exactly recomputes the
few rows whose top-2 gap is below a threshold (device sqrt is a ~1e-4-abs
spline approx; ambiguous rows are patched from f32-exact host values so
the argmax matches the reference bit-for-bit).
"""

import numpy as np

N, D, K = 65536, 128, 1024
NCORES = 8
R = N // NCORES            # rows per core
P = 128                    # partitions / rows per tile
SUB = 8                    # row-tiles per DMA chunk
DECAY = 0.99
COMMIT = 0.25
GAP_THRESHOLD = 1.5e-2     # host-correct rows with top2 gap below this
GSCALE = 512.0             # gumbel fixed-point scale (int16 = g * GSCALE)

# leading columns of the z = g - dist subtraction done on DVE (rest on GPSIMD)
ZSPLIT = 1024

_CACHE = {}
TRACE = False          # set by test harness to collect a HW profile
VERBOSE = False        # print correction statistics
LAST_RESULTS = None    # BassKernelResults of the last run (for profiling)


def _build_nc(rows, loops=1):
    import concourse.bass as bass
    import concourse.mybir as mybir
    from concourse.tile import TileContext

    # --- walrus workaround: split tail-drain waits into single-wait NOPs ---
    from concourse.tile import TileContext as _TC
    from concourse.vector_clock import ScopedClock as _SC

    def _patched_drain(self, tick_clock, wait_clock):
        nc = self.nc
        probe = nc.sync.nop()
        wait_clock.add_sem_waits(probe.ins, _SC({None: tick_clock.global_clock}))
        si = probe.ins.sync_info
        waits = list(si.on_wait or []) if si is not None else []
        if si is not None:
            probe.ins.sync_info = mybir.SyncInfo(
                on_wait=waits[:1], on_update=list(si.on_update or [])
            )
        for w in waits[1:]:
            n2 = nc.sync.nop()
            n2.ins.sync_info = mybir.SyncInfo(on_wait=[w], on_update=[])
        nc.sync.drain()
        nc.all_engine_barrier()
        assert self.sems is not None
        popped = nc._tile_sem_poison_stack.pop()
        assert popped is self._sem_poison
        nc.clear_and_free_semaphores(list(self.sems.allocated().values()))
        nc.all_engine_barrier()

    _TC._drain_and_barrier = _patched_drain

    # --- walrus workaround #2: this compiler rejects instructions carrying
    # more than MAX_WAITS sync waits; hoist the excess onto same-engine NOPs
    # placed immediately before the instruction. ---
    MAX_WAITS = 1

    def _split_excess_waits(nc):
        n_split = 0
        for bbb in nc.bb_map.values():
            insts = bbb.bb.instructions
            out = []
            changed = False
            for inst in insts:
                si = getattr(inst, "sync_info", None)
                waits = list(si.on_wait or []) if si is not None else []
                if len(waits) > MAX_WAITS:
                    extra, keep = waits[:-MAX_WAITS], waits[-MAX_WAITS:]
                    for j in range(0, len(extra), MAX_WAITS):
                        nop = mybir.InstNoOp(
                            name=f"{inst.name}-wsplit-{j}",
                            sync_info=mybir.SyncInfo(
                                on_wait=extra[j:j + MAX_WAITS], on_update=[]
                            ),
                            engine=inst.engine,
                            bass_nofuse=True,
                        )
                        nc.register_instruction(nop, overwrite=True)
                        out.append(nop)
                        n_split += 1
                    inst.sync_info = mybir.SyncInfo(
                        on_wait=keep, on_update=list(si.on_update or [])
                    )
                    changed = True
                out.append(inst)
            if changed:
                insts.clear()
                insts.extend(out)
        return n_split

    f32 = mybir.dt.float32
    f32r = mybir.dt.float32r
    bf16 = mybir.dt.bfloat16
    i16 = mybir.dt.int16
    T = rows // P              # row tiles
    NCH = T // SUB             # DMA chunks

    nc = bass.Bass()
    x_d = nc.declare_dram_parameter("x", [P, T, D], bf16, isOutput=False)
    xt_d = nc.declare_dram_parameter("xt", [D, rows], f32r, isOutput=False)
    x2_d = nc.declare_dram_parameter("x2", [P, T], f32, isOutput=False)
    g_d = nc.declare_dram_parameter("g", [P, NCH, SUB, K], i16, isOutput=False)
    cbt_d = nc.declare_dram_parameter("cbt", [D, K], f32r, isOutput=False)
    c2h_d = nc.declare_dram_parameter("c2h", [1, K], f32r, isOutput=False)
    ones_d = nc.declare_dram_parameter("ones", [1, P], f32r, isOutput=False)
    codes_d = nc.declare_dram_parameter("codes", [P, NCH, SUB, K], bf16, isOutput=True)
    cbsum_d = nc.declare_dram_parameter("cbsumT", [D, K], f32, isOutput=True)
    maxs_d = nc.declare_dram_parameter("maxs", [P, 2 * T], f32, isOutput=True)


    with TileContext(nc) as tc:
        with (
            tc.tile_pool(name="persist", bufs=1) as pp,
            tc.tile_pool(name="gpool", bufs=3) as gp,
            tc.tile_pool(name="cpool", bufs=3) as cp,
            tc.tile_pool(name="work", bufs=4) as wp,
            tc.tile_pool(name="pd", bufs=3, space="PSUM") as pdp,
            tc.tile_pool(name="pacc", bufs=1, space="PSUM") as pap,
        ):
            # persistent loads
            x_sb = pp.tile([P, T, D], bf16)
            xt_sb = pp.tile([D, rows], f32r)
            for c in range(NCH):
                csl = slice(c * SUB * P, (c + 1) * SUB * P)
                nc.sync.dma_start(out=xt_sb[:, csl], in_=xt_d[:, csl])
                nc.sync.dma_start(
                    out=x_sb[:, c * SUB:(c + 1) * SUB, :],
                    in_=x_d[:, c * SUB:(c + 1) * SUB, :])
            x2_sb = pp.tile([P, T], f32)
            nc.sync.dma_start(out=x2_sb[:], in_=x2_d[:])
            cbt_sb = pp.tile([D, K], f32r)
            nc.sync.dma_start(out=cbt_sb[:], in_=cbt_d[:])
            c2h_sb = pp.tile([1, K], f32r)
            nc.sync.dma_start(out=c2h_sb[:], in_=c2h_d[:])
            onesrow = pp.tile([1, P], f32r)
            nc.sync.dma_start(out=onesrow[:], in_=ones_d[:])
            maxs_sb = pp.tile([P, 2 * T], f32)

            pcbT = pap.tile([D, K], f32)        # x^T @ codes accumulator (cbsum^T)

            for it in range(loops):
              for c in range(NCH):
                g_sb = gp.tile([P, SUB, K], i16, tag="g")
                nc.sync.dma_start(out=g_sb[:], in_=g_d[:, c, :, :])
                codes_sb = cp.tile([P, SUB, K], bf16, tag="codes")
                for s in range(SUB):
                    t = c * SUB + s
                    pd = pdp.tile([P, K], f32, tag="pd")
                    for h in range(2):
                        ksl = slice(h * 512, (h + 1) * 512)
                        nc.tensor.matmul(
                            pd[:, ksl],
                            lhsT=xt_sb[:, t * P:(t + 1) * P],
                            rhs=cbt_sb[:, ksl],
                            start=True, stop=False,
                        )
                        nc.tensor.matmul(
                            pd[:, ksl],
                            lhsT=onesrow[:1, :],
                            rhs=c2h_sb[:1, ksl],
                            start=False, stop=True,
                        )
                    # dist = sqrt(x2 - 2*pd)   (pd = x.c - 0.5*||c||^2)
                    dist = wp.tile([P, K], i16, tag="dist")
                    nc.scalar.activation(
                        dist[:], pd[:],
                        mybir.ActivationFunctionType.Sqrt,
                        bias=x2_sb[:, t:t + 1], scale=-2.0 * GSCALE * GSCALE,
                    )
                    # z = g - dist  (split DVE / GPSIMD)
                    z = wp.tile([P, K], i16, tag="z")
                    gs = g_sb[:, s, :]
                    if ZSPLIT > 0:
                        nc.vector.tensor_sub(z[:, :ZSPLIT], gs[:, :ZSPLIT], dist[:, :ZSPLIT])
                    if ZSPLIT < K:
                        nc.gpsimd.tensor_sub(z[:, ZSPLIT:], gs[:, ZSPLIT:], dist[:, ZSPLIT:])
                    # top-8 per row
                    m8 = wp.tile([P, 8], f32, tag="m8")
                    nc.vector.max(out=m8[:], in_=z[:])
                    nc.scalar.copy(out=maxs_sb[:, 2 * t:2 * t + 2], in_=m8[:, 0:2])
                    # one-hot: codes = (z == rowmax)
                    nc.vector.tensor_scalar(
                        codes_sb[:, s, :], z[:], m8[:, 0:1], None,
                        op0=mybir.AluOpType.is_equal,
                    )
                    # accumulate x^T @ codes
                    for h in range(2):
                        ksl = slice(h * 512, (h + 1) * 512)
                        nc.tensor.matmul(
                            pcbT[:, ksl], lhsT=x_sb[:, t, :],
                            rhs=codes_sb[:, s, ksl],
                            start=(t == 0), stop=(t == T - 1),
                        )
                nc.sync.dma_start(out=codes_d[:, c, :, :], in_=codes_sb[:])

            cbs_sb = pp.tile([D, K], f32)
            nc.scalar.copy(out=cbs_sb[:], in_=pcbT[:])
            nc.sync.dma_start(out=cbsum_d[:], in_=cbs_sb[:])
            nc.sync.dma_start(out=maxs_d[:], in_=maxs_sb[:])

    _split_excess_waits(nc)
    return nc


def _host_prep(inputs_np, codebook_np):
    """Host-side constant/derived tensors. Gumbel must match
    jax.random.gumbel(key(42), (N,K), float32) bit-for-bit."""
    if "g" not in _CACHE:
        import jax
        import jax.numpy as jnp
        g = np.asarray(jax.random.gumbel(jax.random.key(42), (N, K), jnp.float32))
        _CACHE["g"] = g
    g = _CACHE["g"]
    x = np.ascontiguousarray(inputs_np, dtype=np.float32)
    cb = np.ascontiguousarray(codebook_np, dtype=np.float32)
    cbt = np.ascontiguousarray(cb.T)
    c2 = (cb.astype(np.float64) ** 2).sum(1)
    c2h = np.ascontiguousarray((-0.5 * c2).astype(np.float32)[None, :])
    x2 = (x.astype(np.float64) ** 2).sum(1).astype(np.float32)
    return g, x, cb, cbt, c2, c2h, x2


def build_inmaps(x, g, cbt, c2h, x2):
    import ml_dtypes
    T = R // P
    NCH = T // SUB
    g_re = _CACHE.get("g_re")
    if g_re is None:
        g_i16 = np.clip(np.rint(g * GSCALE), -32767, 32767).astype(np.int16)
        # (N,K) -> per core (P, NCH, SUB, K) with row n = c*SUB*P + s*P + p
        g_re = np.ascontiguousarray(
            g_i16.reshape(NCORES, NCH, SUB, P, K).transpose(0, 3, 1, 2, 4))
        _CACHE["g_re"] = g_re
    x_re = np.ascontiguousarray(
        x.reshape(NCORES, T, P, D).transpose(0, 2, 1, 3)).astype(ml_dtypes.bfloat16)
    x2s = (x2.astype(np.float64) * GSCALE * GSCALE).astype(np.float32)
    in_maps = []
    for i in range(NCORES):
        rs = slice(i * R, (i + 1) * R)
        in_maps.append({
            "x": x_re[i],
            "xt": np.ascontiguousarray(x[rs].T),
            "x2": np.ascontiguousarray(x2s[rs].reshape(T, P).T),
            "g": g_re[i],
            "cbt": cbt,
            "c2h": c2h,
            "ones": np.ones((1, P), dtype=np.float32),
        })
    return in_maps


def get_nc():
    if "nc" not in _CACHE:
        _CACHE["nc"] = _build_nc(R)
    return _CACHE["nc"]


def kernel(inputs, codebook):
    from concourse.bass_utils import run_bass_kernel_spmd

    g, x, cb, cbt, c2, c2h, x2 = _host_prep(np.asarray(inputs), np.asarray(codebook))
    T = R // P
    nc = get_nc()
    in_maps = build_inmaps(x, g, cbt, c2h, x2)

    res = run_bass_kernel_spmd(nc, in_maps, list(range(NCORES)), trace=TRACE)
    global LAST_RESULTS
    LAST_RESULTS = res

    codes = np.empty((N, K), dtype=np.float32)
    cbsum = np.zeros((K, D), dtype=np.float64)
    gaps = np.empty(N, dtype=np.float32)
    for i in range(NCORES):
        r = res.results[i]
        rs = slice(i * R, (i + 1) * R)
        codes[rs] = r["codes"].transpose(1, 2, 0, 3).reshape(R, K).astype(np.float32)
        cbsum += r["cbsumT"].T.astype(np.float64)
        m = r["maxs"].reshape(P, T, 2)
        gaps[rs] = (m[:, :, 0] - m[:, :, 1]).T.reshape(R) / np.float32(GSCALE)

    # ---- exact host correction of near-tie rows ----
    x64 = x.astype(np.float64)
    cb64 = cb.astype(np.float64)
    fix = np.flatnonzero(gaps < GAP_THRESHOLD)
    n_flip = 0
    for n in fix:
        xr = x[n].astype(np.float64)
        d2r = (x2[n].astype(np.float64)
               + c2
               - 2.0 * (cb64 @ xr))
        zr = g[n].astype(np.float64) - np.sqrt(d2r)
        k_new = int(np.argmax(zr))
        row = codes[n]
        nz = np.flatnonzero(row)
        if len(nz) == 1 and nz[0] == k_new:
            continue
        n_flip += 1
        # undo whatever the device accumulated for this row, then redo
        cbsum -= row[:, None].astype(np.float64) * xr[None, :]
        row[:] = 0.0
        row[k_new] = 1.0
        cbsum[k_new] += xr

    if VERBOSE:
        print(f"[kernel] corrected rows checked: {len(fix)}, flipped: {n_flip}")

    # ---- assemble scalar outputs ----
    counts = codes.sum(0, dtype=np.float64)
    sum_x2 = float((x64 ** 2).sum())
    sum_d2_sel = sum_x2 + float(counts @ c2) - 2.0 * float((cbsum * cb64).sum())
    loss = np.float32((1.0 + COMMIT) * sum_d2_sel / (N * D))
    new_cb = (DECAY * cb64 + (1.0 - DECAY) * cbsum).astype(np.float32)
    return codes, loss, new_cb
